# revision 1
# baseline (speedup 1.0000x reference)
"""Trainium2 Bass kernel for an LSTM cell (DPLSTMCell).

  gates = input @ W_ih^T + b_ih + h_0 @ W_hh^T + b_hh          [B, 4H]
  i, f, g, o = split(gates, 4)
  c_1 = sigmoid(f) * c_0 + sigmoid(i) * tanh(g)
  h_1 = sigmoid(o) * tanh(c_1)

B=16384, IN=H=1024. Data-parallel across 8 NeuronCores: each core gets a
2048-row batch shard; weights/biases are replicated.

Host-side prep (inside kernel()): operands the TensorE needs transposed are
pre-transposed and cast to bf16 on the host (x^T / h^T tiles, W^T in a
matmul-friendly 5D layout, b_ih+b_hh combined). The device then runs pure
bf16 matmuls with fp32 PSUM accumulation plus an fp32 sigmoid/tanh epilogue.

Device layout per core:
  xT4 [128, MT, KT, 128] bf16 : xT4[p, m, kt, b] = x[m*128 + b, kt*128 + p]
  wih5 [128, KT, 2, 4, 512] bf16 : wih5[p, kt, j, q, s] = W_ih[q*1024 + j*512 + s, kt*128 + p]
  bias [1, 2, 4, 512] fp32 : bias[0, j, q, s] = (b_ih + b_hh)[q*1024 + j*512 + s]
  c0 / h1 / c1 [2048, 1024] fp32 natural.

Per batch-tile m (128 rows) and gate-column group j (512 of 1024 columns):
4 PSUM banks (i, f, g, o) each accumulate 16 k-tile matmuls of
[128k x 128b]^T @ [128k x 512g]; the fp32 bias (DMA-broadcast across
partitions once) is added on DVE during the PSUM->SBUF move, then ACT
applies sigmoid/tanh and DVE forms c_1 / h_1.
"""

import os
import sys

import numpy as np

for _p in ("/opt/trn_rl_repo", "/root/.axon_site/_ro/trn_rl_repo"):
    if os.path.isdir(_p) and _p not in sys.path:
        sys.path.append(_p)

import ml_dtypes  # noqa: E402

import concourse.bass as bass  # noqa: E402
import concourse.mybir as mybir  # noqa: E402
import concourse.tile as tile  # noqa: E402
from concourse.bass_utils import run_bass_kernel_spmd  # noqa: E402

N_CORES = 8
B = 16384
IN = 1024
H = 1024
BL = B // N_CORES  # 2048 rows per core
MT = BL // 128     # 16 batch tiles per core
KT = IN // 128     # 8 k-tiles
NQ = 512           # free dim per PSUM bank
BF16 = ml_dtypes.bfloat16

# The walrus in this container only accepts one sync-wait command per
# instruction; Tile emits instructions (notably the final drain) with more.
_MAX_WAITS_PER_INST = 1


def _split_excess_waits(nc, cap=_MAX_WAITS_PER_INST):
    """Move excess sem-waits onto NoOps inserted ahead of the instruction
    (same engine). Waits are AND-conditions on monotonically increasing
    semaphores, so satisfying them one-by-one is equivalent."""
    for f in nc.m.functions:
        for blk in f.blocks:
            new_insts = []
            for inst in blk.instructions:
                si = getattr(inst, "sync_info", None)
                if si is not None and si.on_wait and len(si.on_wait) > cap:
                    waits = list(si.on_wait)
                    extra, keep = waits[:-cap], waits[-cap:]
                    while extra:
                        chunk, extra = extra[:cap], extra[cap:]
                        new_insts.append(
                            mybir.InstNoOp(
                                name=nc.get_next_instruction_name(),
                                sync_info=mybir.SyncInfo(on_wait=chunk, on_update=[]),
                                bass_nofuse=True,
                                engine=inst.engine,
                            )
                        )
                    inst.sync_info = mybir.SyncInfo(
                        on_wait=keep, on_update=list(si.on_update or [])
                    )
                new_insts.append(inst)
            blk.instructions[:] = new_insts


def _build_nc(repeat=None):
    """repeat>1 wraps the whole body in a hardware loop — benchmarking only
    (outputs are simply rewritten each iteration)."""
    if repeat is None:
        repeat = int(os.environ.get("LSTM_BENCH_REPEAT", "1"))
    nc = bass.Bass()
    f32 = mybir.dt.float32
    bf16 = mybir.dt.bfloat16
    SIG = mybir.ActivationFunctionType.Sigmoid
    TANH = mybir.ActivationFunctionType.Tanh

    xT4 = nc.declare_dram_parameter("xT4", [128, MT, KT, 128], bf16, isOutput=False)
    hT4 = nc.declare_dram_parameter("hT4", [128, MT, KT, 128], bf16, isOutput=False)
    c0 = nc.declare_dram_parameter("c0", [BL, H], f32, isOutput=False)
    wih5 = nc.declare_dram_parameter("wih5", [128, KT, 2, 4, NQ], bf16, isOutput=False)
    whh5 = nc.declare_dram_parameter("whh5", [128, KT, 2, 4, NQ], bf16, isOutput=False)
    dve_bias = os.environ.get("LSTM_DVE_BIAS", "1") == "1"
    if dve_bias:
        bjqs = nc.declare_dram_parameter("bjqs", [1, 2, 4, NQ], f32, isOutput=False)
    else:
        bjqs_bf = nc.declare_dram_parameter(
            "bjqs_bf", [1, 2, 4, NQ], bf16, isOutput=False
        )
    h1 = nc.declare_dram_parameter("h1", [BL, H], f32, isOutput=True)
    c1 = nc.declare_dram_parameter("c1", [BL, H], f32, isOutput=True)

    with tile.TileContext(nc) as tc:
        with (
            tc.tile_pool(name="w", bufs=1) as wpool,
            tc.tile_pool(name="xh", bufs=4) as xhpool,
            tc.tile_pool(name="cc", bufs=4) as cpool,
            tc.tile_pool(name="act", bufs=2) as apool,
            tc.tile_pool(name="outp", bufs=4) as opool,
            tc.tile_pool(name="ps", bufs=8, space="PSUM") as pspool,
        ):
            wih_sb = wpool.tile([128, KT, 2, 4, NQ], bf16)
            whh_sb = wpool.tile([128, KT, 2, 4, NQ], bf16)
            if dve_bias:
                bias_sb = wpool.tile([128, 2, 4, NQ], f32)
            else:
                bias_bf = wpool.tile([128, 2, 4, NQ], bf16)
                ones_sb = wpool.tile([128, 128], bf16)
                nc.vector.memset(bias_bf, 0.0)
                nc.vector.memset(ones_sb, 0.0)
                nc.vector.memset(ones_sb[0:1, :], 1.0)

            if repeat > 1:
                loop_cm = tc.For_i(0, repeat, 1)
                loop_cm.__enter__()

            # Weights on the SP HWDGE queue in exact consumption order (the
            # scheduler runs each PSUM bank's chain x-kt0..7 then h-kt0..7):
            # 0.5 MiB chunks so the first bank's matmuls start ~3x sooner.
            # x/h/c0 loads go on the ACT queue; outputs on SP after weights.
            if os.environ.get("LSTM_COARSE_WDMA", "0") == "1":
                for j in range(2):
                    for q in range(4):
                        nc.sync.dma_start(out=wih_sb[:, :, j, q], in_=wih5[:, :, j, q])
                        nc.sync.dma_start(out=whh_sb[:, :, j, q], in_=whh5[:, :, j, q])
            else:
                for j in range(2):
                    for q in range(4):
                        for w_sb, w_dr in ((wih_sb, wih5), (whh_sb, whh5)):
                            for kh in range(2):
                                ks = slice(kh * 4, (kh + 1) * 4)
                                nc.sync.dma_start(
                                    out=w_sb[:, ks, j, q], in_=w_dr[:, ks, j, q]
                                )

            for m in range(MT):
                xm = xhpool.tile([128, KT, 128], bf16, tag="xm")
                hm = xhpool.tile([128, KT, 128], bf16, tag="hm")
                nc.scalar.dma_start(out=xm, in_=xT4[:, m])
                nc.scalar.dma_start(out=hm, in_=hT4[:, m])
                if m == 0:
                    # bias isn't needed until the first matmul group finishes;
                    # keep it behind the first x/h tiles on the ACT queue.
                    if dve_bias:
                        bj_ap = bjqs[:]
                        bias_bcast = bass.AP(
                            tensor=bj_ap.tensor,
                            offset=bj_ap.offset,
                            ap=[[0, 128]] + list(bj_ap.ap[1:]),
                        )
                        nc.scalar.dma_start(out=bias_sb, in_=bias_bcast)
                    else:
                        nc.scalar.dma_start(out=bias_bf[0:1], in_=bjqs_bf[:])
                for j in range(2):
                    cs = slice(j * NQ, (j + 1) * NQ)
                    rs = slice(m * 128, (m + 1) * 128)

                    c0t = cpool.tile([128, NQ], f32, tag="c0")
                    nc.scalar.dma_start(out=c0t, in_=c0[rs, cs])

                    ps = [
                        pspool.tile([128, NQ], f32, tag="ps", name=f"ps{q}")
                        for q in range(4)
                    ]
                    # The scheduler chains these per PSUM bank, which lets each
                    # bank's epilogue overlap the remaining banks' matmuls.
                    if not dve_bias:
                        for q in range(4):
                            nc.tensor.matmul(
                                ps[q], lhsT=ones_sb, rhs=bias_bf[:, j, q],
                                start=True, stop=False, skip_group_check=True,
                            )
                    for kt in range(KT):
                        for q in range(4):
                            nc.tensor.matmul(
                                ps[q], lhsT=xm[:, kt], rhs=wih_sb[:, kt, j, q],
                                start=(dve_bias and kt == 0), stop=False,
                                skip_group_check=True,
                            )
                    for kt in range(KT):
                        last = kt == KT - 1
                        for q in range(4):
                            nc.tensor.matmul(
                                ps[q], lhsT=hm[:, kt], rhs=whh_sb[:, kt, j, q],
                                start=False, stop=last, skip_group_check=True,
                            )

                    gi = apool.tile([128, NQ], f32, tag="gi")
                    gf = apool.tile([128, NQ], f32, tag="gf")
                    gg = apool.tile([128, NQ], f32, tag="gg")
                    go = apool.tile([128, NQ], f32, tag="go")
                    if dve_bias:
                        # bias add on DVE (PSUM -> SBUF), then ACT in place
                        nc.vector.tensor_add(out=gi, in0=ps[0], in1=bias_sb[:, j, 0])
                        nc.vector.tensor_add(out=gf, in0=ps[1], in1=bias_sb[:, j, 1])
                        nc.vector.tensor_add(out=gg, in0=ps[2], in1=bias_sb[:, j, 2])
                        nc.vector.tensor_add(out=go, in0=ps[3], in1=bias_sb[:, j, 3])
                        nc.scalar.activation(out=gi, in_=gi, func=SIG)
                        nc.scalar.activation(out=gf, in_=gf, func=SIG)
                        nc.scalar.activation(out=gg, in_=gg, func=TANH)
                        nc.scalar.activation(out=go, in_=go, func=SIG)
                    else:
                        # bias was seeded into PSUM by the ones-row matmul
                        nc.scalar.activation(out=gi, in_=ps[0], func=SIG)
                        nc.scalar.activation(out=gf, in_=ps[1], func=SIG)
                        nc.scalar.activation(out=gg, in_=ps[2], func=TANH)
                        nc.scalar.activation(out=go, in_=ps[3], func=SIG)

                    nc.vector.tensor_mul(out=gi, in0=gi, in1=gg)   # sig(i)*tanh(g)
                    nc.vector.tensor_mul(out=gf, in0=gf, in1=c0t)  # sig(f)*c0
                    c1t = opool.tile([128, NQ], f32, tag="c1")
                    nc.vector.tensor_add(out=c1t, in0=gi, in1=gf)
                    tc1 = apool.tile([128, NQ], f32, tag="tc1")
                    nc.scalar.activation(out=tc1, in_=c1t, func=TANH)
                    h1t = opool.tile([128, NQ], f32, tag="h1")
                    nc.vector.tensor_mul(out=h1t, in0=go, in1=tc1)

                    nc.sync.dma_start(out=c1[rs, cs], in_=c1t)
                    nc.sync.dma_start(out=h1[rs, cs], in_=h1t)

            if repeat > 1:
                loop_cm.__exit__(None, None, None)

    _split_excess_waits(nc)
    if os.environ.get("LSTM_LDW_DEDUPE", "0") == "1":
        _dedupe_ldweights(nc)
    return nc


def _dedupe_ldweights(nc):
    """Remove an InstLdweights whose weights AP matches the previous
    InstLdweights on PE, with only InstMatmult in between — the PE array
    still holds those weights, so the reload is redundant. Only drops
    instructions with no semaphore waits/updates."""
    n = 0
    for f in nc.m.functions:
        for blk in f.blocks:
            prev_key = None
            keep = []
            for inst in blk.instructions:
                if getattr(inst, "engine", None) != mybir.EngineType.PE:
                    keep.append(inst)
                    continue
                tn = type(inst).__name__
                if tn == "InstLdweights":
                    w = inst.ins[0]
                    key = (w.memref, w.offset, str(w.ap), str(w.dtype))
                    si = getattr(inst, "sync_info", None)
                    clean = si is None or (not si.on_wait and not si.on_update)
                    if key == prev_key and clean:
                        n += 1
                        continue  # drop it
                    prev_key = key
                elif tn != "InstMatmult":
                    prev_key = None
                keep.append(inst)
            blk.instructions[:] = keep
    return n


_NC = None


def _get_nc():
    global _NC
    if _NC is None:
        _NC = _build_nc()
    return _NC


def _prep_xT4(x):
    """[B, 1024] fp32 -> [N_CORES][128, MT, KT, 128] bf16 per-core arrays."""
    v = x.reshape(N_CORES, MT, 128, KT, 128)  # [c, m, b, kt, p]
    v = v.transpose(0, 4, 1, 3, 2)            # [c, p, m, kt, b]
    v = v.astype(BF16)
    return [np.ascontiguousarray(v[c]) for c in range(N_CORES)]


def _prep_w5(w):
    """[4096, 1024] fp32 -> [128, KT, 2, 4, 512] bf16 (replicated)."""
    v = w.reshape(4, 2, NQ, KT, 128)  # [q, j, s, kt, p]
    v = v.transpose(4, 3, 1, 0, 2)    # [p, kt, j, q, s]
    return np.ascontiguousarray(v.astype(BF16))


def _make_in_maps(input, h_0, c_0, W_ih, b_ih, W_hh, b_hh):
    x = np.asarray(input, dtype=np.float32)
    h0 = np.asarray(h_0, dtype=np.float32)
    c0 = np.asarray(c_0, dtype=np.float32)
    wih = np.asarray(W_ih, dtype=np.float32)
    whh = np.asarray(W_hh, dtype=np.float32)
    b = (np.asarray(b_ih, dtype=np.float32) + np.asarray(b_hh, dtype=np.float32))

    xs = _prep_xT4(x)
    hs = _prep_xT4(h0)
    wih5 = _prep_w5(wih)
    whh5 = _prep_w5(whh)
    bjqs = np.ascontiguousarray(
        b.reshape(4, 2, NQ).transpose(1, 0, 2)[None].astype(np.float32)
    )  # [1, 2(j), 4(q), 512]
    bjqs_bf = np.ascontiguousarray(bjqs.astype(BF16))
    c0s = c0.reshape(N_CORES, BL, H)

    return [
        {
            "xT4": xs[c],
            "hT4": hs[c],
            "c0": np.ascontiguousarray(c0s[c]),
            "wih5": wih5,
            "whh5": whh5,
            "bjqs": bjqs,
            "bjqs_bf": bjqs_bf,
        }
        for c in range(N_CORES)
    ]


def kernel(input, h_0, c_0, W_ih, b_ih, W_hh, b_hh):
    in_maps = _make_in_maps(input, h_0, c_0, W_ih, b_ih, W_hh, b_hh)
    nc = _get_nc()
    res = run_bass_kernel_spmd(nc, in_maps, core_ids=list(range(N_CORES)))
    h_1 = np.concatenate([res.results[c]["h1"] for c in range(N_CORES)], axis=0)
    c_1 = np.concatenate([res.results[c]["c1"] for c in range(N_CORES)], axis=0)
    return (h_1, c_1)

